# revision 9
# baseline (speedup 1.0000x reference)
"""Trainium2 Bass kernel for a 4-step differentiable recurrent net forward pass.

Reference computation (B=8192, NI=512, NH=2048, NO=512, 4 steps):
    activs = 0; outputs = 0
    repeat 4x:  pre = hr * (x @ Wih.T + activs @ Whh.T + outputs @ Woh.T) + hb
                activs = per_neuron_act(pre)        # tanh/sigmoid/relu by i%3
    out = sigmoid(or * (x @ Wio.T + outputs @ Woo.T + activs @ Who.T) + ob)

`outputs` is never written inside the loop, so the Woh/Woo terms vanish and
the x-projection P = hr*(x@Wih.T)+hb is loop-invariant (computed once).

Strategy: data-parallel on batch across 8 cores (1024 rows each). On-core
everything is feature-major (features on SBUF partitions, batch on the free
axis), so each matmul is W_tile.T @ X^T with stationary bf16 weights.
Host-side prep: hidden neurons are permuted so the three activation groups
are contiguous (per-partition-range ACTs instead of masked blends), hr/or
are folded into the weight matrices, and hb/ob are folded in via an extra
K=1 matmul against a ones row. Compute is bf16 with f32 PSUM accumulation.
"""

import os

import numpy as np
import ml_dtypes

import concourse.bass as bass
import concourse.tile as tile
from concourse import bacc, mybir
from concourse.bass_utils import run_bass_kernel_spmd

B, NI, NH, NO = 8192, 512, 2048, 512
N_STEPS = 4
N_CORES = 8
BL = B // N_CORES          # batch rows per core
CH = 512                   # batch chunk (one PSUM bank of fp32)
NCH = BL // CH             # 2 chunks per core
KI = NI // 128             # 4 k-tiles over inputs
KH = NH // 128             # 16 k/m-tiles over hidden
KO = NO // 128             # 4 m-tiles over outputs

BF16 = mybir.dt.bfloat16
F32 = mybir.dt.float32
AF = mybir.ActivationFunctionType

# hidden neurons regrouped as [all tanh | all sigmoid | all relu]
_idx = np.arange(NH)
PERM = np.concatenate([_idx[_idx % 3 == 0], _idx[_idx % 3 == 1], _idx[_idx % 3 == 2]])
_B1 = int((_idx % 3 == 0).sum())           # 683
_B2 = _B1 + int((_idx % 3 == 1).sum())     # 1366

# per m-tile: the single activation function, or None for the two mixed tiles
_TILE_FUNC = []
for _m in range(KH):
    _lo, _hi = _m * 128, (_m + 1) * 128
    _fs = set()
    for _f, _a, _b in ((AF.Tanh, 0, _B1), (AF.Sigmoid, _B1, _B2), (AF.Relu, _B2, NH)):
        if max(_lo, _a) < min(_hi, _b):
            _fs.add(_f)
    _TILE_FUNC.append(_fs.pop() if len(_fs) == 1 else None)

# mixed tiles: (major_func applied everywhere, minor_func, mask column block)
# partition sub-ranges must be 32-aligned on TRN2, so the minority strip is
# fixed up with a full-tile ACT + copy_predicated against a {0,1} mask
_BOUNDARY = {
    _B1 // 128: (AF.Sigmoid, AF.Tanh, 0),    # tile 5: parts < 43 are tanh
    _B2 // 128: (AF.Sigmoid, AF.Relu, 1),    # tile 10: parts >= 86 are relu
}


def _emit_hidden_act(nc, ps, blk, a_new, tmp_pool, bmask_t, bias=None):
    """Evict a 4-bank PSUM block through the grouped activations into a_new.

    ps:    PSUM AP (128, 4*CH) holding m-tiles blk*4..blk*4+3 (one per bank)
    a_new: SBUF AP (128, KH*CH) bf16, m-tile m lives at [:, m*CH:(m+1)*CH]
    bias:  optional (128, KH) f32 SBUF tile of per-partition biases; forces
           per-tile ACTs (used on step 1, where PSUM lacks the hidden bias)
    """
    mloc = 0
    while mloc < 4:
        m = blk * 4 + mloc
        bias_ap = bias[:, m:m + 1] if bias is not None else 0.0
        if m in _BOUNDARY:
            major, minor, mb = _BOUNDARY[m]
            nc.scalar.activation(
                a_new[:, m * CH:(m + 1) * CH],
                ps[:, mloc * CH:(mloc + 1) * CH], major, bias=bias_ap)
            t = tmp_pool.tile([128, CH], BF16, tag="btmp", bufs=2, name="btmp")
            nc.scalar.activation(t[:], ps[:, mloc * CH:(mloc + 1) * CH], minor,
                                 bias=bias_ap)
            nc.vector.copy_predicated(
                a_new[:, m * CH:(m + 1) * CH],
                bmask_t[:, mb * CH:(mb + 1) * CH], t[:])
            mloc += 1
            continue
        func = _TILE_FUNC[m]
        end = mloc + 1
        if bias is None:
            while end < 4 and _TILE_FUNC[blk * 4 + end] == func:
                end += 1
        nc.scalar.activation(
            a_new[:, (blk * 4 + mloc) * CH:(blk * 4 + end) * CH],
            ps[:, mloc * CH:end * CH], func, bias=bias_ap)
        mloc = end


def _build_nc():
    nc = bacc.Bacc("TRN2", target_bir_lowering=False, debug=False,
                   num_devices=N_CORES)

    xT = nc.dram_tensor("xT", [NI, BL], BF16, kind="ExternalInput").ap()
    wih = nc.dram_tensor("wih", [NI, NH], BF16, kind="ExternalInput").ap()
    whh = nc.dram_tensor("whh", [NH, NH], BF16, kind="ExternalInput").ap()
    who = nc.dram_tensor("who", [NO, KO * NO], BF16, kind="ExternalInput").ap()
    wio = nc.dram_tensor("wio", [NI, NO], BF16, kind="ExternalInput").ap()
    hbc = nc.dram_tensor("hbc", [128, KH], F32, kind="ExternalInput").ap()
    obc = nc.dram_tensor("obc", [128, KO], F32, kind="ExternalInput").ap()
    bmask = nc.dram_tensor("bmask", [128, 2 * CH], mybir.dt.uint8,
                           kind="ExternalInput").ap()
    outT = nc.dram_tensor("outT", [NO, BL], F32, kind="ExternalOutput").ap()

    with tile.TileContext(nc) as tc:
        with tc.tile_pool(name="w", bufs=1) as wpool, \
             tc.tile_pool(name="act", bufs=1) as apool, \
             tc.tile_pool(name="ps", bufs=2, space="PSUM") as pspool, \
             tc.tile_pool(name="out", bufs=4) as opool:

            # ---- stage inputs (emission order ~= DMA priority) ----
            xT_t = []
            wih_t = []
            for k in range(KI):
                w = wpool.tile([128, NH], BF16, tag=f"proj{k}", name=f"wih{k}")
                nc.sync.dma_start(w[:], wih[k * 128:(k + 1) * 128, :])
                wih_t.append(w)
                t = wpool.tile([128, BL], BF16, tag=f"x{k}", name=f"x{k}")
                nc.sync.dma_start(t[:, 0:CH], xT[k * 128:(k + 1) * 128, 0:CH])
                xT_t.append(t)
            for k in range(KI):
                nc.sync.dma_start(xT_t[k][:, CH:BL],
                                  xT[k * 128:(k + 1) * 128, CH:BL])
            hbc_t = wpool.tile([128, KH], F32, tag="hbc")
            nc.sync.dma_start(hbc_t[:], hbc[:])
            obc_t = wpool.tile([128, KO], F32, tag="obc")
            nc.sync.dma_start(obc_t[:], obc[:])
            bmask_t = wpool.tile([128, 2 * CH], mybir.dt.uint8, tag="bmask")
            nc.sync.dma_start(bmask_t[:], bmask[:])
            whh_t = []
            for k in range(KH):
                t = wpool.tile([128, NH], BF16, tag=f"whh{k}", name=f"whh{k}")
                eng = nc.sync if k % 2 == 0 else nc.scalar
                eng.dma_start(t[:], whh[k * 128:(k + 1) * 128, :])
                whh_t.append(t)
            wio_t = []
            for k in range(KI):
                t = wpool.tile([128, NO], BF16, tag=f"wio{k}", name=f"wio{k}")
                nc.scalar.dma_start(t[:], wio[k * 128:(k + 1) * 128, :])
                wio_t.append(t)

            # ---- per-chunk x-projection P and first-step activations ----
            P = {}
            A = {}
            for c in range(NCH):
                P[c] = apool.tile([128, KH * CH], BF16, tag=f"P{c}",
                                  name=f"P{c}")
                a1 = apool.tile([128, KH * CH], BF16, tag="A", bufs=3,
                                name=f"A1c{c}")
                for blk in range(4):
                    ps = pspool.tile([128, 4 * CH], F32, tag="ps", name="psb")
                    for mloc in range(4):
                        m = blk * 4 + mloc
                        oap = ps[:, mloc * CH:(mloc + 1) * CH]
                        for k in range(KI):
                            nc.tensor.matmul(
                                oap, wih_t[k][:, m * 128:(m + 1) * 128],
                                xT_t[k][:, c * CH:(c + 1) * CH],
                                start=(k == 0), stop=(k == KI - 1))
                    for mloc in range(4):
                        m = blk * 4 + mloc
                        nc.vector.tensor_scalar_add(
                            P[c][:, m * CH:(m + 1) * CH],
                            ps[:, mloc * CH:(mloc + 1) * CH],
                            hbc_t[:, m:m + 1])
                    _emit_hidden_act(nc, ps, blk, a1, opool, bmask_t,
                                     bias=hbc_t)
                A[c] = a1

            # ---- recurrent steps 2..4 ----
            for s in range(N_STEPS - 1):
                for c in range(NCH):
                    a_new = apool.tile([128, KH * CH], BF16, tag="A", bufs=3,
                                       name=f"A{s + 2}c{c}")
                    for blk in range(4):
                        ps = pspool.tile([128, 4 * CH], F32, tag="ps",
                                         name="psb")
                        for mloc in range(4):
                            m = blk * 4 + mloc
                            oap = ps[:, mloc * CH:(mloc + 1) * CH]
                            for k in range(KH):
                                nc.tensor.matmul(
                                    oap, whh_t[k][:, m * 128:(m + 1) * 128],
                                    A[c][:, k * CH:(k + 1) * CH],
                                    start=(k == 0), stop=(k == KH - 1))
                        nc.vector.tensor_add(
                            ps[:], ps[:],
                            P[c][:, blk * 4 * CH:(blk + 1) * 4 * CH])
                        _emit_hidden_act(nc, ps, blk, a_new, opool, bmask_t)
                    A[c] = a_new

            # ---- output layer (who reuses the wih slots) ----
            who_t = []
            for j in range(KO):
                t = wpool.tile([128, NH], BF16, tag=f"proj{j}", name=f"who{j}")
                nc.scalar.dma_start(t[:], who[j * 128:(j + 1) * 128, :])
                who_t.append(t)
            for c in range(NCH):
                ps = pspool.tile([128, 4 * CH], F32, tag="ps", name="psb")
                for mo in range(KO):
                    oap = ps[:, mo * CH:(mo + 1) * CH]
                    for k in range(KI):
                        nc.tensor.matmul(
                            oap, wio_t[k][:, mo * 128:(mo + 1) * 128],
                            xT_t[k][:, c * CH:(c + 1) * CH],
                            start=(k == 0), stop=False)
                    for kk in range(KH):
                        j, sj = divmod(kk, KO)
                        nc.tensor.matmul(
                            oap,
                            who_t[j][:, sj * NO + mo * 128:
                                     sj * NO + (mo + 1) * 128],
                            A[c][:, kk * CH:(kk + 1) * CH],
                            start=False, stop=(kk == KH - 1))
                for mo in range(KO):
                    o = opool.tile([128, CH], F32, tag="o", name="o")
                    nc.scalar.activation(
                        o[:], ps[:, mo * CH:(mo + 1) * CH], AF.Sigmoid,
                        bias=obc_t[:, mo:mo + 1])
                    nc.sync.dma_start(
                        outT[mo * 128:(mo + 1) * 128, c * CH:(c + 1) * CH],
                        o[:])

    nc.compile()
    return nc


_NC_CACHE = None


def _get_nc():
    global _NC_CACHE
    if _NC_CACHE is None:
        _NC_CACHE = _build_nc()
    return _NC_CACHE


def _make_bmask():
    m = np.zeros((128, 2 * CH), np.uint8)
    m[:_B1 - (_B1 // 128) * 128, 0:CH] = 1          # tile 5: parts < 43 tanh
    m[_B2 - (_B2 // 128) * 128:, CH:2 * CH] = 1     # tile 10: parts >= 86 relu
    return m


def _prep_in_maps(inputs):
    bf = ml_dtypes.bfloat16
    x = np.asarray(inputs["inputs"], np.float32)
    hr = np.asarray(inputs["hidden_responses"], np.float32)[PERM]
    hb = np.asarray(inputs["hidden_biases"], np.float32)[PERM]
    orr = np.asarray(inputs["output_responses"], np.float32)
    ob = np.asarray(inputs["output_biases"], np.float32)

    wih_s = (hr[:, None] * np.asarray(inputs["input_to_hidden"], np.float32)[PERM]).T
    whh_s = (hr[:, None] *
             np.asarray(inputs["hidden_to_hidden"], np.float32)[PERM][:, PERM]).T
    who_s = (orr[:, None] *
             np.asarray(inputs["hidden_to_output"], np.float32)[:, PERM]).T
    wio_s = (orr[:, None] * np.asarray(inputs["input_to_output"], np.float32)).T
    # pack who (NH, NO) so SBUF tile j holds k-tiles j*4..j*4+3 side by side
    who_p = np.ascontiguousarray(
        who_s.reshape(KO, KO, 128, NO).transpose(0, 2, 1, 3).reshape(NO, KO * NO))

    shared = {
        "wih": np.ascontiguousarray(wih_s).astype(bf),
        "whh": np.ascontiguousarray(whh_s).astype(bf),
        "who": who_p.astype(bf),
        "wio": np.ascontiguousarray(wio_s).astype(bf),
        "hbc": np.ascontiguousarray(hb.reshape(KH, 128).T),
        "obc": np.ascontiguousarray(ob.reshape(KO, 128).T),
        "bmask": _make_bmask(),
    }
    in_maps = []
    for c in range(N_CORES):
        m = dict(shared)
        m["xT"] = np.ascontiguousarray(x[c * BL:(c + 1) * BL].T).astype(bf)
        in_maps.append(m)
    return in_maps


def _run(inputs, trace=False, tmpdir=None):
    nc = _get_nc()
    in_maps = _prep_in_maps(inputs)
    res = run_bass_kernel_spmd(nc, in_maps, core_ids=list(range(N_CORES)),
                               trace=trace, tmpdir=tmpdir)
    out = np.empty((B, NO), np.float32)
    for c in range(N_CORES):
        out[c * BL:(c + 1) * BL] = res.results[c]["outT"].T
    return out, res


def kernel(**inputs) -> np.ndarray:
    out, _ = _run(inputs, trace=False)
    return out


if __name__ == "__main__":
    rng = np.random.default_rng(0)
    ins = {
        "inputs": rng.standard_normal((B, NI), dtype=np.float32),
        "input_to_hidden": rng.standard_normal((NH, NI), dtype=np.float32) * 0.02,
        "hidden_to_hidden": rng.standard_normal((NH, NH), dtype=np.float32) * 0.02,
        "output_to_hidden": rng.standard_normal((NH, NO), dtype=np.float32) * 0.02,
        "input_to_output": rng.standard_normal((NO, NI), dtype=np.float32) * 0.02,
        "hidden_to_output": rng.standard_normal((NO, NH), dtype=np.float32) * 0.02,
        "output_to_output": rng.standard_normal((NO, NO), dtype=np.float32) * 0.02,
        "hidden_responses": rng.standard_normal(NH, dtype=np.float32) * 0.1 + 1.0,
        "hidden_biases": rng.standard_normal(NH, dtype=np.float32) * 0.1,
        "output_responses": rng.standard_normal(NO, dtype=np.float32) * 0.1 + 1.0,
        "output_biases": rng.standard_normal(NO, dtype=np.float32) * 0.1,
    }
    out = kernel(**ins)
    print("kernel output", out.shape, out.dtype, out[:2, :4])


# revision 10
# speedup vs baseline: 1.0152x; 1.0152x over previous
"""Trainium2 Bass kernel for a 4-step differentiable recurrent net forward pass.

Reference computation (B=8192, NI=512, NH=2048, NO=512, 4 steps):
    activs = 0; outputs = 0
    repeat 4x:  pre = hr * (x @ Wih.T + activs @ Whh.T + outputs @ Woh.T) + hb
                activs = per_neuron_act(pre)        # tanh/sigmoid/relu by i%3
    out = sigmoid(or * (x @ Wio.T + outputs @ Woo.T + activs @ Who.T) + ob)

`outputs` is never written inside the loop, so the Woh/Woo terms vanish and
the x-projection P = hr*(x@Wih.T)+hb is loop-invariant (computed once).

Strategy: data-parallel on batch across 8 cores (1024 rows each). On-core
everything is feature-major (features on SBUF partitions, batch on the free
axis), so each matmul is W_tile.T @ X^T with stationary bf16 weights.
Host-side prep: hidden neurons are permuted so the three activation groups
are contiguous (per-partition-range ACTs instead of masked blends), hr/or
are folded into the weight matrices, and hb/ob are folded in via an extra
K=1 matmul against a ones row. Compute is bf16 with f32 PSUM accumulation.
"""

import os

import numpy as np
import ml_dtypes

import concourse.bass as bass
import concourse.tile as tile
from concourse import bacc, mybir
from concourse.bass_utils import run_bass_kernel_spmd

B, NI, NH, NO = 8192, 512, 2048, 512
N_STEPS = 4
N_CORES = 8
BL = B // N_CORES          # batch rows per core
CH = 512                   # batch chunk (one PSUM bank of fp32)
NCH = BL // CH             # 2 chunks per core
KI = NI // 128             # 4 k-tiles over inputs
KH = NH // 128             # 16 k/m-tiles over hidden
KO = NO // 128             # 4 m-tiles over outputs

BF16 = mybir.dt.bfloat16
F32 = mybir.dt.float32
AF = mybir.ActivationFunctionType

# hidden neurons regrouped as [all tanh | all sigmoid | all relu]
_idx = np.arange(NH)
PERM = np.concatenate([_idx[_idx % 3 == 0], _idx[_idx % 3 == 1], _idx[_idx % 3 == 2]])
_B1 = int((_idx % 3 == 0).sum())           # 683
_B2 = _B1 + int((_idx % 3 == 1).sum())     # 1366

# per m-tile: the single activation function, or None for the two mixed tiles
_TILE_FUNC = []
for _m in range(KH):
    _lo, _hi = _m * 128, (_m + 1) * 128
    _fs = set()
    for _f, _a, _b in ((AF.Tanh, 0, _B1), (AF.Sigmoid, _B1, _B2), (AF.Relu, _B2, NH)):
        if max(_lo, _a) < min(_hi, _b):
            _fs.add(_f)
    _TILE_FUNC.append(_fs.pop() if len(_fs) == 1 else None)

# mixed tiles: (major_func applied everywhere, minor_func, mask column block)
# partition sub-ranges must be 32-aligned on TRN2, so the minority strip is
# fixed up with a full-tile ACT + copy_predicated against a {0,1} mask
_BOUNDARY = {
    _B1 // 128: (AF.Sigmoid, AF.Tanh, 0),    # tile 5: parts < 43 are tanh
    _B2 // 128: (AF.Sigmoid, AF.Relu, 1),    # tile 10: parts >= 86 are relu
}


def _emit_hidden_act(nc, ps, blk, a_new, tmp_pool, bmask_t, bias=None):
    """Evict a 4-bank PSUM block through the grouped activations into a_new.

    ps:    PSUM AP (128, 4*CH) holding m-tiles blk*4..blk*4+3 (one per bank)
    a_new: SBUF AP (128, KH*CH) bf16, m-tile m lives at [:, m*CH:(m+1)*CH]
    bias:  optional (128, KH) f32 SBUF tile of per-partition biases; forces
           per-tile ACTs (used on step 1, where PSUM lacks the hidden bias)
    """
    mloc = 0
    while mloc < 4:
        m = blk * 4 + mloc
        bias_ap = bias[:, m:m + 1] if bias is not None else 0.0
        if m in _BOUNDARY:
            major, minor, mb = _BOUNDARY[m]
            nc.scalar.activation(
                a_new[:, m * CH:(m + 1) * CH],
                ps[:, mloc * CH:(mloc + 1) * CH], major, bias=bias_ap)
            t = tmp_pool.tile([128, CH], BF16, tag="btmp", bufs=2, name="btmp")
            nc.scalar.activation(t[:], ps[:, mloc * CH:(mloc + 1) * CH], minor,
                                 bias=bias_ap)
            nc.vector.copy_predicated(
                a_new[:, m * CH:(m + 1) * CH],
                bmask_t[:, mb * CH:(mb + 1) * CH], t[:])
            mloc += 1
            continue
        func = _TILE_FUNC[m]
        end = mloc + 1
        if bias is None:
            while end < 4 and _TILE_FUNC[blk * 4 + end] == func:
                end += 1
        nc.scalar.activation(
            a_new[:, (blk * 4 + mloc) * CH:(blk * 4 + end) * CH],
            ps[:, mloc * CH:end * CH], func, bias=bias_ap)
        mloc = end


def _build_nc():
    nc = bacc.Bacc("TRN2", target_bir_lowering=False, debug=False,
                   num_devices=N_CORES)

    xT = nc.dram_tensor("xT", [NI, BL], BF16, kind="ExternalInput").ap()
    wih = nc.dram_tensor("wih", [NI, NH], BF16, kind="ExternalInput").ap()
    whh = nc.dram_tensor("whh", [NH, NH], BF16, kind="ExternalInput").ap()
    who = nc.dram_tensor("who", [NO, KO * NO], BF16, kind="ExternalInput").ap()
    wio = nc.dram_tensor("wio", [NI, NO], BF16, kind="ExternalInput").ap()
    hbc = nc.dram_tensor("hbc", [128, KH], F32, kind="ExternalInput").ap()
    obc = nc.dram_tensor("obc", [128, KO], F32, kind="ExternalInput").ap()
    bmask = nc.dram_tensor("bmask", [128, 2 * CH], mybir.dt.uint8,
                           kind="ExternalInput").ap()
    outT = nc.dram_tensor("outT", [NO, BL], F32, kind="ExternalOutput").ap()

    with tile.TileContext(nc) as tc:
        with tc.tile_pool(name="w", bufs=1) as wpool, \
             tc.tile_pool(name="act", bufs=1) as apool, \
             tc.tile_pool(name="ps", bufs=2, space="PSUM") as pspool, \
             tc.tile_pool(name="out", bufs=4) as opool:

            # ---- stage inputs ----
            # two HWDGE queues (SP ~2x faster than ACT here); wih + most of
            # whh ride SP, x/wio/who + a few whh tiles ride ACT
            wih_t = []
            for k in range(KI):
                w = wpool.tile([128, NH], BF16, tag=f"proj{k}", name=f"wih{k}")
                nc.sync.dma_start(w[:], wih[k * 128:(k + 1) * 128, :])
                wih_t.append(w)
            xT_t = []
            for k in range(KI):
                t = wpool.tile([128, BL], BF16, tag=f"x{k}", name=f"x{k}")
                nc.scalar.dma_start(t[:, 0:CH], xT[k * 128:(k + 1) * 128, 0:CH])
                xT_t.append(t)
            for k in range(KI):
                nc.scalar.dma_start(xT_t[k][:, CH:BL],
                                    xT[k * 128:(k + 1) * 128, CH:BL])
            hbc_t = wpool.tile([128, KH], F32, tag="hbc")
            nc.sync.dma_start(hbc_t[:], hbc[:])
            obc_t = wpool.tile([128, KO], F32, tag="obc")
            nc.sync.dma_start(obc_t[:], obc[:])
            bmask_t = wpool.tile([128, 2 * CH], mybir.dt.uint8, tag="bmask")
            nc.sync.dma_start(bmask_t[:], bmask[:])
            wio_t = []
            for k in range(KI):
                t = wpool.tile([128, NO], BF16, tag=f"wio{k}", name=f"wio{k}")
                nc.scalar.dma_start(t[:], wio[k * 128:(k + 1) * 128, :])
                wio_t.append(t)
            whh_t = []
            for k in range(KH):
                t = wpool.tile([128, NH], BF16, tag=f"whh{k}", name=f"whh{k}")
                eng = nc.scalar if k % 4 == 3 else nc.sync
                eng.dma_start(t[:], whh[k * 128:(k + 1) * 128, :])
                whh_t.append(t)

            # ---- per-chunk x-projection P and first-step activations ----
            P = {}
            A = {}
            for c in range(NCH):
                P[c] = apool.tile([128, KH * CH], BF16, tag=f"P{c}",
                                  name=f"P{c}")
                a1 = apool.tile([128, KH * CH], BF16, tag="A", bufs=3,
                                name=f"A1c{c}")
                for blk in range(4):
                    ps = pspool.tile([128, 4 * CH], F32, tag="ps", name="psb")
                    for k in range(KI):
                        for mloc in range(4):
                            m = blk * 4 + mloc
                            nc.tensor.matmul(
                                ps[:, mloc * CH:(mloc + 1) * CH],
                                wih_t[k][:, m * 128:(m + 1) * 128],
                                xT_t[k][:, c * CH:(c + 1) * CH],
                                start=(k == 0), stop=(k == KI - 1))
                    for mloc in range(4):
                        m = blk * 4 + mloc
                        nc.vector.tensor_scalar_add(
                            P[c][:, m * CH:(m + 1) * CH],
                            ps[:, mloc * CH:(mloc + 1) * CH],
                            hbc_t[:, m:m + 1])
                    _emit_hidden_act(nc, ps, blk, a1, opool, bmask_t,
                                     bias=hbc_t)
                A[c] = a1

            # ---- whh-independent output x-projection (fills the window
            # while the 8MB whh load is still in flight) ----
            outx = {}
            for c in range(NCH):
                outx[c] = apool.tile([128, KO * CH], F32, tag=f"outx{c}",
                                     name=f"outx{c}")
                ps = pspool.tile([128, 4 * CH], F32, tag="ps", name="psb")
                for k in range(KI):
                    for mo in range(KO):
                        nc.tensor.matmul(
                            ps[:, mo * CH:(mo + 1) * CH],
                            wio_t[k][:, mo * 128:(mo + 1) * 128],
                            xT_t[k][:, c * CH:(c + 1) * CH],
                            start=(k == 0), stop=(k == KI - 1))
                nc.vector.tensor_copy(outx[c][:], ps[:])

            # ---- recurrent steps 2..4 ----
            for s in range(N_STEPS - 1):
                for c in range(NCH):
                    a_new = apool.tile([128, KH * CH], BF16, tag="A", bufs=3,
                                       name=f"A{s + 2}c{c}")
                    for blk in range(4):
                        ps = pspool.tile([128, 4 * CH], F32, tag="ps",
                                         name="psb")
                        for k in range(KH):
                            for mloc in range(4):
                                m = blk * 4 + mloc
                                nc.tensor.matmul(
                                    ps[:, mloc * CH:(mloc + 1) * CH],
                                    whh_t[k][:, m * 128:(m + 1) * 128],
                                    A[c][:, k * CH:(k + 1) * CH],
                                    start=(k == 0), stop=(k == KH - 1))
                        nc.vector.tensor_add(
                            ps[:], ps[:],
                            P[c][:, blk * 4 * CH:(blk + 1) * 4 * CH])
                        _emit_hidden_act(nc, ps, blk, a_new, opool, bmask_t)
                    A[c] = a_new

            # ---- output layer (who reuses the wih slots) ----
            who_t = []
            for j in range(KO):
                t = wpool.tile([128, NH], BF16, tag=f"proj{j}", name=f"who{j}")
                nc.scalar.dma_start(t[:], who[j * 128:(j + 1) * 128, :])
                who_t.append(t)
            for c in range(NCH):
                ps = pspool.tile([128, 4 * CH], F32, tag="ps", name="psb")
                for kk in range(KH):
                    j, sj = divmod(kk, KO)
                    for mo in range(KO):
                        nc.tensor.matmul(
                            ps[:, mo * CH:(mo + 1) * CH],
                            who_t[j][:, sj * NO + mo * 128:
                                     sj * NO + (mo + 1) * 128],
                            A[c][:, kk * CH:(kk + 1) * CH],
                            start=(kk == 0), stop=(kk == KH - 1))
                nc.vector.tensor_add(ps[:], ps[:], outx[c][:])
                for mo in range(KO):
                    o = opool.tile([128, CH], F32, tag="o", name="o")
                    nc.scalar.activation(
                        o[:], ps[:, mo * CH:(mo + 1) * CH], AF.Sigmoid,
                        bias=obc_t[:, mo:mo + 1])
                    nc.sync.dma_start(
                        outT[mo * 128:(mo + 1) * 128, c * CH:(c + 1) * CH],
                        o[:])

    nc.compile()
    return nc


_NC_CACHE = None


def _get_nc():
    global _NC_CACHE
    if _NC_CACHE is None:
        _NC_CACHE = _build_nc()
    return _NC_CACHE


def _make_bmask():
    m = np.zeros((128, 2 * CH), np.uint8)
    m[:_B1 - (_B1 // 128) * 128, 0:CH] = 1          # tile 5: parts < 43 tanh
    m[_B2 - (_B2 // 128) * 128:, CH:2 * CH] = 1     # tile 10: parts >= 86 relu
    return m


def _prep_in_maps(inputs):
    bf = ml_dtypes.bfloat16
    x = np.asarray(inputs["inputs"], np.float32)
    hr = np.asarray(inputs["hidden_responses"], np.float32)[PERM]
    hb = np.asarray(inputs["hidden_biases"], np.float32)[PERM]
    orr = np.asarray(inputs["output_responses"], np.float32)
    ob = np.asarray(inputs["output_biases"], np.float32)

    wih_s = (hr[:, None] * np.asarray(inputs["input_to_hidden"], np.float32)[PERM]).T
    whh_s = (hr[:, None] *
             np.asarray(inputs["hidden_to_hidden"], np.float32)[PERM][:, PERM]).T
    who_s = (orr[:, None] *
             np.asarray(inputs["hidden_to_output"], np.float32)[:, PERM]).T
    wio_s = (orr[:, None] * np.asarray(inputs["input_to_output"], np.float32)).T
    # pack who (NH, NO) so SBUF tile j holds k-tiles j*4..j*4+3 side by side
    who_p = np.ascontiguousarray(
        who_s.reshape(KO, KO, 128, NO).transpose(0, 2, 1, 3).reshape(NO, KO * NO))

    shared = {
        "wih": np.ascontiguousarray(wih_s).astype(bf),
        "whh": np.ascontiguousarray(whh_s).astype(bf),
        "who": who_p.astype(bf),
        "wio": np.ascontiguousarray(wio_s).astype(bf),
        "hbc": np.ascontiguousarray(hb.reshape(KH, 128).T),
        "obc": np.ascontiguousarray(ob.reshape(KO, 128).T),
        "bmask": _make_bmask(),
    }
    in_maps = []
    for c in range(N_CORES):
        m = dict(shared)
        m["xT"] = np.ascontiguousarray(x[c * BL:(c + 1) * BL].T).astype(bf)
        in_maps.append(m)
    return in_maps


def _run(inputs, trace=False, tmpdir=None):
    nc = _get_nc()
    in_maps = _prep_in_maps(inputs)
    res = run_bass_kernel_spmd(nc, in_maps, core_ids=list(range(N_CORES)),
                               trace=trace, tmpdir=tmpdir)
    out = np.empty((B, NO), np.float32)
    for c in range(N_CORES):
        out[c * BL:(c + 1) * BL] = res.results[c]["outT"].T
    return out, res


def kernel(**inputs) -> np.ndarray:
    out, _ = _run(inputs, trace=False)
    return out


if __name__ == "__main__":
    rng = np.random.default_rng(0)
    ins = {
        "inputs": rng.standard_normal((B, NI), dtype=np.float32),
        "input_to_hidden": rng.standard_normal((NH, NI), dtype=np.float32) * 0.02,
        "hidden_to_hidden": rng.standard_normal((NH, NH), dtype=np.float32) * 0.02,
        "output_to_hidden": rng.standard_normal((NH, NO), dtype=np.float32) * 0.02,
        "input_to_output": rng.standard_normal((NO, NI), dtype=np.float32) * 0.02,
        "hidden_to_output": rng.standard_normal((NO, NH), dtype=np.float32) * 0.02,
        "output_to_output": rng.standard_normal((NO, NO), dtype=np.float32) * 0.02,
        "hidden_responses": rng.standard_normal(NH, dtype=np.float32) * 0.1 + 1.0,
        "hidden_biases": rng.standard_normal(NH, dtype=np.float32) * 0.1,
        "output_responses": rng.standard_normal(NO, dtype=np.float32) * 0.1 + 1.0,
        "output_biases": rng.standard_normal(NO, dtype=np.float32) * 0.1,
    }
    out = kernel(**ins)
    print("kernel output", out.shape, out.dtype, out[:2, :4])
